# revision 46
# baseline (speedup 1.0000x reference)
"""Trainium2 Bass kernel for the VQ commitment-loss problem (fp8 DoubleRow).

Math
----
reference loss = 0.25 * mean((codebook[argmin_k dist] - flat)**2)
               = 0.25/(B*T*D) * sum_n min_k ||flat_n - e_k||^2
since the gathered quantized row realizes exactly the min squared distance.

min_k ||f - e||^2 = ||f||^2 + min_k (||e_k||^2 - 2 f.e_k)

Per core (2 of 16 batches):
  - sum_n ||f_n||^2 via the window-count trick:
        sum over tau of cnt(tau) * x_pad[tau]^2   (cnt = #windows containing tau)
  - the min term via fp8e4 DoubleRow TensorE matmuls (256-deep contraction
    per pass): window tiles [128, 4sub, T] are the stationary operand, the
    codebook scaled by -2 is the moving operand [128, 4sub, 1024].
    ||e_k||^2 rides as three extra contraction rows (32*r0 + r1 + r2 fp8
    decomposition, paired with a 32/1/1 column in the window operand).
    Two 128-window subtiles share a 4-bank PSUM tile; one VectorE 3D
    min-reduce [128,2,1024] -> [128,2] drains it.

All window data is expanded up-front into two resident [128, 4, 4096] SBUF
tiles through the SWDGE (gpsimd) queue — it spreads packets over all 16
SDMA engines, unlike the dynamic HWDGE rings which only engage ~3 for this
pattern — in staged waves so the main loop starts early.  Single-queue FIFO
keeps DMA completion monotone so shared completion-semaphore lanes cannot
alias a later DMA into an early matmul's wait.

Host side only pads/casts/shards inputs and sums the 8 per-core partials.
"""

import numpy as np
import ml_dtypes

B, P, T = 16, 12, 4096
WIN = 41
PAD = (WIN - 1) // 2          # 20
K = 1024
D = P * WIN                   # 492
COMMITMENT_COST = 0.25

NCORES = 8
BC = B // NCORES              # batches per core = 2
TP = T + 2 * PAD              # padded time = 4136
NCHUNK = 4                    # contraction subtiles: 3 pellets * 41 taps = 123 rows
CHROWS = 3 * WIN              # 123
NSUB = BC * T // 128          # 64 subtiles of 128 windows per core
NPAIR = NSUB // 2             # 32 PSUM pair-tiles
TCHUNK = TP // 4              # 1034 (xsq layout)
NWARM = 24                    # HAM warmup matmuls (bridge PE to main-loop start)

SCALE = COMMITMENT_COST / (B * T * D)

FP8NP = ml_dtypes.float8_e4m3

_CACHED = {}


def _build_nc():
    import concourse.bacc as bacc
    import concourse.bass as bass
    import concourse.mybir as mybir
    import concourse.tile as tile

    BF = mybir.dt.bfloat16
    F32 = mybir.dt.float32
    F8 = mybir.dt.float8e4
    AX = mybir.AxisListType
    OP = mybir.AluOpType
    DR = mybir.MatmulPerfMode.DoubleRow

    nc = bacc.Bacc("TRN2", target_bir_lowering=False, debug=False)

    xw_d = nc.dram_tensor("xw", [BC, P, TP], F8, kind="ExternalInput")
    cb_d = nc.dram_tensor("cb", [128, NCHUNK, K], F8, kind="ExternalInput")
    cnt_d = nc.dram_tensor("cnt", [96, TCHUNK], BF, kind="ExternalInput")
    ones_d = nc.dram_tensor("ones5", [5, T], F8, kind="ExternalInput")
    out_d = nc.dram_tensor("out", [1, 1], F32, kind="ExternalOutput")

    with tile.TileContext(nc) as tc:
        with (
            tc.tile_pool(name="cbpool", bufs=1) as cbpool,
            tc.tile_pool(name="wpool", bufs=1) as wpool,
            tc.tile_pool(name="misc", bufs=1) as misc,
        ):
            # ---- HAM warmup: PE busy from t~0 so the clock is 2.4 GHz when
            # the real matmuls start.
            warm_src = misc.tile([128, 512], BF)
            nc.vector.memset(warm_src[:], 0.5)
            with tc.tile_pool(name="pwarm", bufs=1, space="PSUM") as pwarm:
                wps = pwarm.tile([128, 512], F32)
                for _ in range(NWARM):
                    nc.tensor.matmul(
                        wps[:], warm_src[:, 0:128], warm_src[:], start=True, stop=True
                    )

            # ---- resident codebook tile [k, subtile, code]
            cbt = cbpool.tile([128, NCHUNK, K], F8)
            nc.gpsimd.dma_start(cbt[:], cb_d[:])

            ones_bf = misc.tile([128, 1], BF)
            nc.vector.memset(ones_bf[:], 1.0)
            ones_f = misc.tile([128, 1], F32)
            nc.vector.memset(ones_f[:], 1.0)
            mins_buf = misc.tile([128, NSUB], F32)

            # ---- resident window tiles wt[b]: [128, sub, T] fp8 with
            # wt[b][k, c, t] = xw[b, 3c + k//41, t + k%41] for k < 123.
            wt = [
                wpool.tile([128, NCHUNK, T], F8, tag=f"w{b}", name=f"wt{b}")
                for b in range(BC)
            ]

            def wslice_dma(c, b, lo, hi):
                nc.gpsimd.dma_start(
                    wt[b][0:CHROWS, c, lo:hi],
                    bass.AP(
                        xw_d,
                        (b * P + 3 * c) * TP + lo,
                        [[TP, 3], [1, WIN], [1, hi - lo]],
                    ),
                )

            # ones rows for every window tile, on the scalar ring up front
            # (rows 123..127 get [32, 1, 1, 1, 1] from the host constant)
            for b in range(BC):
                for c in range(NCHUNK):
                    nc.scalar.dma_start(wt[b][CHROWS:128, c, :], ones_d[:])

            # batch-0 expansion waves (after cb on the same ring)
            for lo, hi in ((0, 1024), (1024, 2048), (2048, 3072), (3072, T)):
                for c in range(NCHUNK):
                    wslice_dma(c, 0, lo, hi)

            # ---- prologue: c_k = ||e_k||^2 as fp8 rows 32*r0 + r1 + r2
            # into cbt rows 123..125 of subtile 0
            with (
                tc.tile_pool(name="pre", bufs=1) as pre,
                tc.tile_pool(name="ppre", bufs=1, space="PSUM") as ppre,
            ):
                sq = pre.tile([128, NCHUNK, K], BF)
                nc.vector.tensor_mul(sq[:], cbt[:], cbt[:])  # (-2e)^2 = 4 e^2
                cq = pre.tile([1, K], F32)
                for h in range(2):
                    pc = ppre.tile([1, 512], F32, tag=f"pc{h}", name=f"pc{h}")
                    for c in range(NCHUNK):
                        nc.tensor.matmul(
                            pc[:],
                            ones_bf[:],
                            sq[:, c, 512 * h : 512 * (h + 1)],
                            start=(c == 0),
                            stop=(c == NCHUNK - 1),
                        )
                    nc.vector.tensor_scalar_mul(
                        cq[:, 512 * h : 512 * (h + 1)], pc[:], 0.25
                    )
                r0 = pre.tile([1, K], F8)
                nc.vector.tensor_scalar_mul(r0[:], cq[:], 1.0 / 32.0)
                r0f = pre.tile([1, K], F32)
                nc.vector.tensor_copy(r0f[:], r0[:])
                t1 = pre.tile([1, K], F32)
                nc.vector.tensor_scalar_mul(t1[:], r0f[:], 32.0)
                rem1 = pre.tile([1, K], F32)
                nc.vector.tensor_sub(rem1[:], cq[:], t1[:])
                r1 = pre.tile([1, K], F8)
                nc.vector.tensor_copy(r1[:], rem1[:])
                nc.sync.dma_start(cbt[CHROWS : CHROWS + 1, 0, :], r0[:])
                nc.sync.dma_start(cbt[CHROWS + 1 : CHROWS + 2, 0, :], r1[:])

            # batch-1 expansion waves (gpsimd ring, after batch 0)
            for c in range(NCHUNK):
                wslice_dma(c, 1, 0, 2048)
            for c in range(NCHUNK):
                wslice_dma(c, 1, 2048, T)

            # ---- xsq/cnt loads (scalar ring)
            xsq_in = misc.tile([96, TCHUNK], F8)
            nc.scalar.dma_start(
                xsq_in[:],
                bass.AP(
                    xw_d,
                    0,
                    [[P * TP, BC], [TP, P], [TCHUNK, 4], [1, TCHUNK]],
                ),
            )
            cnt_sb = misc.tile([96, TCHUNK], BF)
            nc.scalar.dma_start(cnt_sb[:], cnt_d[:])

            sqx = misc.tile([96, TCHUNK], BF)
            wsq = misc.tile([96, TCHUNK], F32)
            selfsum = misc.tile([96, 1], F32)

            # ---- main loop: 32 pairs of 128-window subtiles
            with tc.tile_pool(name="pmain", bufs=2, space="PSUM") as pmain:
                for pair in range(NPAIR):
                    if pair == 6:
                        # slot the ||f||^2 term into DVE idle time mid-loop
                        nc.vector.tensor_mul(sqx[:], xsq_in[:], xsq_in[:])
                        nc.vector.tensor_mul(wsq[:], sqx[:], cnt_sb[:])
                        nc.vector.tensor_reduce(
                            selfsum[:], wsq[:], axis=AX.X, op=OP.add
                        )
                    ps = pmain.tile([128, 2, K], F32, tag="ps", name=f"ps_{pair}")
                    for s in range(2):
                        i = pair * 2 + s            # subtile index
                        b = i // (NSUB // BC)
                        toff = (i % (NSUB // BC)) * 128
                        for h in range(2):
                            # subtile pair (0,1) last: it carries the
                            # codebook-norm rows, which are ready latest
                            for jp in (2, 0):
                                nc.tensor.matmul(
                                    ps[:, s, 512 * h : 512 * (h + 1)],
                                    wt[b][:, jp : jp + 2, toff : toff + 128],
                                    cbt[:, jp : jp + 2, 512 * h : 512 * (h + 1)],
                                    start=(jp == 2),
                                    stop=(jp == 0),
                                    perf_mode=DR,
                                )
                    nc.vector.tensor_reduce(
                        mins_buf[:, 2 * pair : 2 * pair + 2],
                        ps[:],
                        axis=AX.X,
                        op=OP.min,
                    )

            # ---- finale: grand sum -> scale -> out
            macc = misc.tile([128, 1], F32)
            nc.vector.tensor_reduce(macc[:], mins_buf[:], axis=AX.X, op=OP.add)
            with tc.tile_pool(name="pfin", bufs=1, space="PSUM") as pfin:
                fin = pfin.tile([1, 1], F32)
                nc.tensor.matmul(fin[:], macc[:], ones_f[:], start=True, stop=False)
                nc.tensor.matmul(
                    fin[:], selfsum[:], ones_f[0:96, :], start=False, stop=True
                )
                res = misc.tile([1, 1], F32)
                nc.vector.tensor_scalar_mul(res[:], fin[:], float(SCALE))
                nc.gpsimd.dma_start(out_d[:], res[:])

    nc.compile()
    return nc


def get_nc():
    if "nc" not in _CACHED:
        _CACHED["nc"] = _build_nc()
    return _CACHED["nc"]


def _host_prep(x, codebook):
    """Pad/cast/shard the inputs; returns per-core in_maps."""
    x = np.asarray(x, dtype=np.float32)
    codebook = np.asarray(codebook, dtype=np.float32)

    x8 = x.astype(FP8NP)
    xw = np.zeros((B, P, TP), dtype=FP8NP)
    xw[:, :, PAD : PAD + T] = x8

    # value of the fp8-rounded codebook, exactly scaled by -2
    cbb = codebook.astype(FP8NP).astype(np.float32)
    rhs = np.zeros((128, NCHUNK, K), dtype=np.float32)
    for c in range(NCHUNK):
        rhs[:CHROWS, c, :] = -2.0 * cbb[:, CHROWS * c : CHROWS * (c + 1)].T
    rhs8 = rhs.astype(FP8NP)

    tau = np.arange(TP, dtype=np.float32)
    cnt = np.minimum(np.minimum(tau + 1.0, float(WIN)), float(TP) - tau)
    cnt_rep = np.tile(cnt.reshape(4, TCHUNK), (BC * P, 1)).astype(ml_dtypes.bfloat16)

    ones5 = np.ones((5, T), dtype=FP8NP)
    ones5[0, :] = FP8NP(32.0)

    in_maps = []
    for i in range(NCORES):
        in_maps.append(
            {
                "xw": np.ascontiguousarray(xw[BC * i : BC * (i + 1)]),
                "cb": rhs8,
                "cnt": cnt_rep,
                "ones5": ones5,
            }
        )
    return in_maps


def kernel(x, codebook):
    from concourse.bass_utils import run_bass_kernel_spmd

    nc = get_nc()
    in_maps = _host_prep(x, codebook)
    res = run_bass_kernel_spmd(nc, in_maps, core_ids=list(range(NCORES)))
    total = np.float64(0.0)
    for r in res.results:
        total += np.float64(r["out"][0, 0])
    return np.array(np.float32(total))
